# revision 16
# baseline (speedup 1.0000x reference)
"""Trainium2 Bass kernel for the GNN attention module
(scatter-mean -> dense+tanh -> attention coefs -> weighted scatter-add),
data-parallel over graphs on 8 NeuronCores.

Self-contained: hardcodes N=2000000, D=64, G=8192, 8 cores.

v4 (baseline structure + three surgical changes):
  - xts (transposed x for the dot-products) is fp8_e4m3: halves the
    second read of x (HBM 66.7MB -> 50.4MB per core-set). fp8
    [128,128] stationaries keep FWL eligibility.
  - sigmoid is applied during PSUM evacuation of the all-window dots
    (ACT reads PSUM, writes bf16 SBUF); the one-hot pick becomes a
    cheap bf16 multiply mp = m * sigma(psD) (exact: m is one-hot).
    Replaces the scalar_tensor_tensor saturation trick + f16 sigmoid.
  - the one-hot m is built on the (otherwise idle) gpsimd engine via a
    broadcast-read is_equal; xts streams on the gpsimd SWDGE queue
    while xs streams on the sync HWDGE queue.
Seg-sum matmuls keep the baseline shape: x as the [128,64] stationary
per block, one-hot as the 32-col moving operand, col-tiled pairs at
(0,0)/(0,64), accumulated in [128, WIN] psum, added into acc [64, GCP].
"""
import os
import numpy as np
from contextlib import ExitStack

import ml_dtypes

P = 128          # partitions / nodes per block
T = 32           # blocks per mega-tile
NT = P * T       # nodes per mega-tile (4096)
WIN = 32         # mega window width (graphs)
D = 64
DP2 = D + 2      # x columns + pad + packed per-block graph offset (b32)
N_FULL = 2_000_000
G_FULL = 8192
CORES = 8
GCP = 1152       # padded local graph count (9 * 128)
NCHUNK = GCP // P

LAST_EXEC_NS = None


# ----------------------------------------------------------------------------
# host-side preprocessing
# ----------------------------------------------------------------------------

def _shard_plan(batch, size, cores):
    counts = np.bincount(batch.astype(np.int64), minlength=size)
    cum = np.concatenate([[0], np.cumsum(counts)])
    n = batch.shape[0]
    gsplit = [0]
    for k in range(1, cores):
        g = int(np.searchsorted(cum, k * n / cores))
        g = max(gsplit[-1] + 1, min(g, size - (cores - k)))
        gsplit.append(g)
    gsplit.append(size)
    nsplit = [int(cum[g]) for g in gsplit]
    return gsplit, nsplit, counts


def _prep_core(x, batch, counts, g0, g1, n0, n1, n_meg):
    nn = n1 - n0
    npad = n_meg * NT
    lg = (batch[n0:n1] - g0).astype(np.int64)
    gc = g1 - g0
    ghost = gc                           # pad nodes get this local graph id
    lg_full = np.full(npad, ghost, dtype=np.int64)
    lg_full[:nn] = lg

    xs_pad = np.zeros((npad, D), dtype=np.float32)
    xs_pad[:nn] = x[n0:n1]

    lgt = lg_full.reshape(n_meg, T, P)            # [t, j, p]
    c0 = np.minimum(lgt[:, 0, 0], GCP - WIN)      # mega window base
    b32 = lgt - c0[:, None, None]
    assert b32.min() >= 0 and b32.max() < WIN, (b32.min(), b32.max())
    assert ghost + 1 <= GCP

    b32 = b32.transpose(0, 2, 1).astype(np.float32)   # [t, p, j]

    xs4 = np.zeros((n_meg, P, T, DP2), dtype=np.float32)
    xs4[:, :, :, :D] = xs_pad.reshape(n_meg, T, P, D).transpose(0, 2, 1, 3)
    xs4 = xs4.astype(ml_dtypes.bfloat16)
    xs4[:, :, :, D + 1] = b32.astype(ml_dtypes.bfloat16)
    xtb = xs_pad.reshape(n_meg, T, P, D).transpose(0, 1, 3, 2)  # [t, j, d, q]
    xts = np.ascontiguousarray(
        xtb.reshape(n_meg, T // 2, 2, D, P).transpose(0, 2, 3, 1, 4)
        .reshape(n_meg, P, (T // 2) * P)
    ).astype(ml_dtypes.bfloat16)

    c0s = np.zeros((1, n_meg), dtype=np.int32)
    c0s[0, :] = c0

    # 1 / max(counts, 1) for the local graphs, expanded to 64 partitions
    lc = np.ones(GCP, dtype=np.float64)
    lc[:gc] = np.maximum(counts[g0:g1], 1)
    inv = (1.0 / lc).astype(np.float32)
    invexp = np.ascontiguousarray(np.broadcast_to(inv, (D, GCP)))
    return {"xs": xs4, "xts": xts, "c0s": c0s, "invc": invexp}, gc


def _host_consts(W):
    iota = np.broadcast_to(
        np.arange(WIN, dtype=np.float32), (P, T, WIN)
    ).astype(ml_dtypes.bfloat16)
    ident = np.eye(P, dtype=np.float32)
    w2 = np.ascontiguousarray(
        np.tile(W.astype(np.float32), (1, 2))
    ).astype(ml_dtypes.bfloat16)          # [64, 128] = [W | W]
    return iota, ident, w2


# ----------------------------------------------------------------------------
# device kernel
# ----------------------------------------------------------------------------

def build_nc(n_meg):
    from concourse import mybir
    import concourse.tile as tile
    import concourse.bacc as bacc
    import concourse.bass as bass

    f32 = mybir.dt.float32
    bf16 = mybir.dt.bfloat16
    fp8 = mybir.dt.float8e4
    i32 = mybir.dt.int32
    AF = mybir.ActivationFunctionType
    ALU = mybir.AluOpType
    ENG = mybir.EngineType

    nc = bacc.Bacc("TRN2", target_bir_lowering=False, debug=False,
                   num_devices=CORES)

    xs = nc.dram_tensor("xs", [n_meg, P, T, DP2], bf16, kind="ExternalInput").ap()
    xts = nc.dram_tensor("xts", [n_meg, P, (T // 2) * P], bf16, kind="ExternalInput").ap()
    c0s = nc.dram_tensor("c0s", [1, n_meg], i32, kind="ExternalInput").ap()
    w2c = nc.dram_tensor("w2c", [D, P], bf16, kind="ExternalInput").ap()
    invc = nc.dram_tensor("invc", [D, GCP], f32, kind="ExternalInput").ap()
    iotac = nc.dram_tensor("iotac", [P, T, WIN], bf16, kind="ExternalInput").ap()
    identc = nc.dram_tensor("identc", [P, P], f32, kind="ExternalInput").ap()
    out = nc.dram_tensor("out", [GCP, D], f32, kind="ExternalOutput").ap()

    with tile.TileContext(nc) as tc, ExitStack() as ctx:
        cpool = ctx.enter_context(tc.tile_pool(name="const", bufs=1))
        px = ctx.enter_context(tc.tile_pool(name="px", bufs=8))
        pxt = ctx.enter_context(tc.tile_pool(name="pxt", bufs=5))
        pbe = ctx.enter_context(tc.tile_pool(name="pbe", bufs=3))
        pm = ctx.enter_context(tc.tile_pool(name="pm", bufs=5))
        pmp = ctx.enter_context(tc.tile_pool(name="pmp", bufs=3))
        psg = ctx.enter_context(tc.tile_pool(name="psg", bufs=3))
        ptg = ctx.enter_context(tc.tile_pool(name="ptg", bufs=3))
        pmid = ctx.enter_context(tc.tile_pool(name="pmid", bufs=2))
        pk = ctx.enter_context(tc.tile_pool(name="pk", bufs=3))
        pp1 = ctx.enter_context(tc.tile_pool(name="pp1", bufs=3, space="PSUM"))
        ppd = ctx.enter_context(tc.tile_pool(name="ppd", bufs=4, space="PSUM"))
        ppw = ctx.enter_context(tc.tile_pool(name="ppw", bufs=1, space="PSUM"))

        iota_sb = cpool.tile([P, T, WIN], bf16)
        nc.sync.dma_start(iota_sb[:], iotac[:])
        ident_sb = cpool.tile([P, P], f32)
        nc.sync.dma_start(ident_sb[:], identc[:])
        w2_sb = cpool.tile([D, P], bf16)
        nc.sync.dma_start(w2_sb[:], w2c[:])
        inv_sb = cpool.tile([D, GCP], f32)
        nc.sync.dma_start(inv_sb[:], invc[:])
        c0_sb = cpool.tile([1, n_meg], i32)
        nc.sync.dma_start(c0_sb[:], c0s[:])

        acc1 = cpool.tile([D, GCP], f32)
        nc.vector.memset(acc1[:], 0.0)
        acc2 = cpool.tile([D, GCP], f32)
        nc.vector.memset(acc2[:], 0.0)

        def c0_of(t, engines):
            return nc.values_load(
                c0_sb[0:1, t:t + 1], engines=engines,
                min_val=0, max_val=GCP - WIN, skip_runtime_bounds_check=True)

        xs_ts = [None] * n_meg
        xt_ts = [None] * n_meg
        m_ts = [None] * n_meg
        cv_ts = [None] * n_meg

        def S1(t):
            xs_t = px.tile([P, T, DP2], bf16, tag="xs")
            nc.sync.dma_start(xs_t[:], xs[t])
            xts_t = pxt.tile([P, T // 2, P], bf16, tag="xts")
            nc.scalar.dma_start(xts_t[:], xts[t])
            b32e = pbe.tile([P, T, WIN], bf16, tag="b32e")
            nc.scalar.copy(b32e[:], xs_t[:, :, D + 1].to_broadcast([P, T, WIN]))
            m = pm.tile([P, T, WIN], bf16, tag="M")
            nc.vector.tensor_tensor(out=m[:], in0=iota_sb[:], in1=b32e[:],
                                    op=ALU.is_equal)
            ps1 = pp1.tile([P, WIN], f32, tag="pacc")
            for jj in range(T // 2):
                nc.tensor.matmul(ps1[0:D, :], lhsT=xs_t[:, 2 * jj, 0:D],
                                 rhs=m[:, 2 * jj, :], tile_position=(0, 0),
                                 start=(jj == 0), stop=(jj == T // 2 - 1))
                nc.tensor.matmul(ps1[D:P, :], lhsT=xs_t[:, 2 * jj + 1, 0:D],
                                 rhs=m[:, 2 * jj + 1, :], tile_position=(0, 64),
                                 start=(jj == 0), stop=(jj == T // 2 - 1))
            cv = c0_of(t, engines=[ENG.DVE])
            a = acc1[:, bass.ds(cv, WIN)]
            nc.vector.tensor_tensor(out=a, in0=a, in1=ps1[0:D, :], op=ALU.add)
            nc.vector.tensor_tensor(out=a, in0=a, in1=ps1[D:P, :], op=ALU.add)
            xs_ts[t], xt_ts[t], m_ts[t], cv_ts[t] = xs_t, xts_t, m, cv

        def MID(t):
            cv = cv_ts[t]
            meanT = pmid.tile([D, WIN], bf16, tag="meanT")
            nc.vector.tensor_tensor(out=meanT[:],
                                    in0=acc1[:, bass.ds(cv, WIN)],
                                    in1=inv_sb[:, bass.ds(cv, WIN)],
                                    op=ALU.mult)
            psW = ppw.tile([P, WIN], f32, tag="mid")
            nc.tensor.matmul(psW[:], lhsT=w2_sb[:], rhs=meanT[:],
                             start=True, stop=True)
            tgwin = ptg.tile([P, 2, WIN], bf16, tag="tgwin")
            nc.vector.memset(tgwin[:], 0.0)
            nc.scalar.activation(tgwin[0:D, 0, :], psW[0:D, :], AF.Tanh)
            nc.scalar.activation(tgwin[D:P, 1, :], psW[D:P, :], AF.Tanh)
            return tgwin

        def DOTS(t, tgwin):
            xts_t, m = xt_ts[t], m_ts[t]
            sg = psg.tile([P, T, WIN], bf16, tag="sg")
            for h in range(2):
                psD = ppd.tile([P, T // 2, WIN], f32, tag="psD")
                for j2 in range(T // 4):
                    jj = h * (T // 4) + j2
                    nc.tensor.matmul(psD[:, 2 * j2:2 * j2 + 2, :],
                                     lhsT=xts_t[:, jj, :],
                                     rhs=tgwin[:, :, :],
                                     start=True, stop=True)
                nc.scalar.activation(sg[:, h * (T // 2):(h + 1) * (T // 2), :],
                                     psD[:], AF.Sigmoid)
            mp = pmp.tile([P, T, WIN], bf16, tag="Mp")
            nc.vector.tensor_tensor(out=mp[:], in0=m[:], in1=sg[:],
                                    op=ALU.mult)
            return mp

        def S2(t, mp):
            xs_t, cv = xs_ts[t], cv_ts[t]
            ps3 = pp1.tile([P, WIN], f32, tag="pacc")
            for jj in range(T // 2):
                nc.tensor.matmul(ps3[0:D, :], lhsT=xs_t[:, 2 * jj, 0:D],
                                 rhs=mp[:, 2 * jj, :], tile_position=(0, 0),
                                 start=(jj == 0), stop=(jj == T // 2 - 1))
                nc.tensor.matmul(ps3[D:P, :], lhsT=xs_t[:, 2 * jj + 1, 0:D],
                                 rhs=mp[:, 2 * jj + 1, :], tile_position=(0, 64),
                                 start=(jj == 0), stop=(jj == T // 2 - 1))
            a = acc2[:, bass.ds(cv_ts[t], WIN)]
            nc.vector.tensor_tensor(out=a, in0=a, in1=ps3[0:D, :], op=ALU.add)
            nc.vector.tensor_tensor(out=a, in0=a, in1=ps3[D:P, :], op=ALU.add)
            xs_ts[t] = xt_ts[t] = m_ts[t] = cv_ts[t] = None

        # ---------------- fused pipeline (lag-2: PE never waits) ------------
        S1(0)
        if n_meg > 1:
            S1(1)
        mp_prev = None
        for t in range(n_meg):
            tgwin = MID(t)
            if t + 2 < n_meg:
                S1(t + 2)
            mp = DOTS(t, tgwin)
            if mp_prev is not None:
                S2(t - 1, mp_prev)
            mp_prev = mp
        S2(n_meg - 1, mp_prev)

        # ---------------- end: transpose acc2 -> out ------------------------
        for c in range(NCHUNK):
            pst = ppw.tile([P, D], f32, tag="mid")
            nc.tensor.transpose(pst[:], acc2[0:D, c * P:(c + 1) * P],
                                ident_sb[0:D, 0:D])
            oc = pk.tile([P, D], f32, tag="oc")
            nc.scalar.copy(oc[:], pst[:])
            nc.gpsimd.dma_start(out[c * P:(c + 1) * P, :], oc[:])

    nc.compile()
    return nc


# ----------------------------------------------------------------------------
# entry point
# ----------------------------------------------------------------------------

_CACHE = {}


def kernel(x, batch, size, W):
    global LAST_EXEC_NS
    from concourse import bass_utils

    x = np.asarray(x, dtype=np.float32)
    batch_np = np.asarray(batch).astype(np.int64)
    W = np.asarray(W, dtype=np.float32)
    size = int(size)
    cores = CORES

    gsplit, nsplit, counts = _shard_plan(batch_np, size, cores)
    max_nodes = max(nsplit[k + 1] - nsplit[k] for k in range(cores))
    n_meg = max(2, -(-max_nodes // NT))

    iota, ident, w2 = _host_consts(W)
    in_maps = []
    gcs = []
    for k in range(cores):
        m, gc = _prep_core(x, batch_np, counts, gsplit[k], gsplit[k + 1],
                           nsplit[k], nsplit[k + 1], n_meg)
        m["w2c"] = w2
        m["iotac"] = iota
        m["identc"] = ident
        in_maps.append(m)
        gcs.append(gc)

    if n_meg not in _CACHE:
        _CACHE[n_meg] = build_nc(n_meg)
    nc = _CACHE[n_meg]

    trace = os.environ.get("BASS_KERNEL_TRACE", "0") == "1"
    res = bass_utils.run_bass_kernel_spmd(nc, in_maps,
                                          core_ids=list(range(cores)),
                                          trace=trace)
    LAST_EXEC_NS = res.exec_time_ns
    outs = [res.results[k]["out"][:gcs[k]] for k in range(cores)]
    full = np.concatenate(outs, axis=0)
    if full.shape[0] < size:
        full = np.concatenate(
            [full, np.zeros((size - full.shape[0], D), np.float32)], axis=0)
    return np.ascontiguousarray(full[:size], dtype=np.float32)


# revision 17
# speedup vs baseline: 1.4290x; 1.4290x over previous
"""Trainium2 Bass kernel for the GNN attention module
(scatter-mean -> dense+tanh -> attention coefs -> weighted scatter-add),
data-parallel over graphs on 8 NeuronCores.

Self-contained: hardcodes N=2000000, D=64, G=8192, 8 cores.

v4 (baseline structure + three surgical changes):
  - xts (transposed x for the dot-products) is fp8_e4m3: halves the
    second read of x (HBM 66.7MB -> 50.4MB per core-set). fp8
    [128,128] stationaries keep FWL eligibility.
  - sigmoid is applied during PSUM evacuation of the all-window dots
    (ACT reads PSUM, writes bf16 SBUF); the one-hot pick becomes a
    cheap bf16 multiply mp = m * sigma(psD) (exact: m is one-hot).
    Replaces the scalar_tensor_tensor saturation trick + f16 sigmoid.
  - the one-hot m is built on the (otherwise idle) gpsimd engine via a
    broadcast-read is_equal; xts streams on the gpsimd SWDGE queue
    while xs streams on the sync HWDGE queue.
Seg-sum matmuls keep the baseline shape: x as the [128,64] stationary
per block, one-hot as the 32-col moving operand, col-tiled pairs at
(0,0)/(0,64), accumulated in [128, WIN] psum, added into acc [64, GCP].
"""
import os
import numpy as np
from contextlib import ExitStack

import ml_dtypes

P = 128          # partitions / nodes per block
T = 32           # blocks per mega-tile
NT = P * T       # nodes per mega-tile (4096)
WIN = 32         # mega window width (graphs)
D = 64
DP2 = D + 2      # x columns + pad + packed per-block graph offset (b32)
N_FULL = 2_000_000
G_FULL = 8192
CORES = 8
GCP = 1152       # padded local graph count (9 * 128)
NCHUNK = GCP // P

LAST_EXEC_NS = None


# ----------------------------------------------------------------------------
# host-side preprocessing
# ----------------------------------------------------------------------------

def _shard_plan(batch, size, cores):
    counts = np.bincount(batch.astype(np.int64), minlength=size)
    cum = np.concatenate([[0], np.cumsum(counts)])
    n = batch.shape[0]
    gsplit = [0]
    for k in range(1, cores):
        g = int(np.searchsorted(cum, k * n / cores))
        g = max(gsplit[-1] + 1, min(g, size - (cores - k)))
        gsplit.append(g)
    gsplit.append(size)
    nsplit = [int(cum[g]) for g in gsplit]
    return gsplit, nsplit, counts


def _prep_core(x, batch, counts, g0, g1, n0, n1, n_meg):
    nn = n1 - n0
    npad = n_meg * NT
    lg = (batch[n0:n1] - g0).astype(np.int64)
    gc = g1 - g0
    ghost = gc                           # pad nodes get this local graph id
    lg_full = np.full(npad, ghost, dtype=np.int64)
    lg_full[:nn] = lg

    xs_pad = np.zeros((npad, D), dtype=np.float32)
    xs_pad[:nn] = x[n0:n1]

    lgt = lg_full.reshape(n_meg, T, P)            # [t, j, p]
    c0 = np.minimum(lgt[:, 0, 0], GCP - WIN)      # mega window base
    b32 = lgt - c0[:, None, None]
    assert b32.min() >= 0 and b32.max() < WIN, (b32.min(), b32.max())
    assert ghost + 1 <= GCP

    b32 = b32.transpose(0, 2, 1).astype(np.float32)   # [t, p, j]

    xs4 = np.zeros((n_meg, P, T, DP2), dtype=np.float32)
    xs4[:, :, :, :D] = xs_pad.reshape(n_meg, T, P, D).transpose(0, 2, 1, 3)
    xs4 = xs4.astype(ml_dtypes.bfloat16)
    xs4[:, :, :, D + 1] = b32.astype(ml_dtypes.bfloat16)
    xtb = xs_pad.reshape(n_meg, T, P, D).transpose(0, 1, 3, 2)  # [t, j, d, q]
    xts = np.ascontiguousarray(
        xtb.reshape(n_meg, T // 2, 2, D, P).transpose(0, 2, 3, 1, 4)
        .reshape(n_meg, P, (T // 2) * P)
    ).astype(ml_dtypes.float8_e4m3)

    c0s = np.zeros((1, n_meg), dtype=np.int32)
    c0s[0, :] = c0

    # 1 / max(counts, 1) for the local graphs, expanded to 64 partitions
    lc = np.ones(GCP, dtype=np.float64)
    lc[:gc] = np.maximum(counts[g0:g1], 1)
    inv = (1.0 / lc).astype(np.float32)
    invexp = np.ascontiguousarray(np.broadcast_to(inv, (D, GCP)))
    return {"xs": xs4, "xts": xts, "c0s": c0s, "invc": invexp}, gc


def _host_consts(W):
    iota = np.broadcast_to(
        np.arange(WIN, dtype=np.float32), (P, T, WIN)
    ).astype(ml_dtypes.bfloat16)
    ident = np.eye(P, dtype=np.float32)
    w2 = np.ascontiguousarray(
        np.tile(W.astype(np.float32), (1, 2))
    ).astype(ml_dtypes.bfloat16)          # [64, 128] = [W | W]
    return iota, ident, w2


# ----------------------------------------------------------------------------
# device kernel
# ----------------------------------------------------------------------------

def build_nc(n_meg):
    from concourse import mybir
    import concourse.tile as tile
    import concourse.bacc as bacc
    import concourse.bass as bass

    f32 = mybir.dt.float32
    bf16 = mybir.dt.bfloat16
    fp8 = mybir.dt.float8e4
    i32 = mybir.dt.int32
    AF = mybir.ActivationFunctionType
    ALU = mybir.AluOpType
    ENG = mybir.EngineType

    nc = bacc.Bacc("TRN2", target_bir_lowering=False, debug=False,
                   num_devices=CORES)

    xs = nc.dram_tensor("xs", [n_meg, P, T, DP2], bf16, kind="ExternalInput").ap()
    xts = nc.dram_tensor("xts", [n_meg, P, (T // 2) * P], fp8, kind="ExternalInput").ap()
    c0s = nc.dram_tensor("c0s", [1, n_meg], i32, kind="ExternalInput").ap()
    w2c = nc.dram_tensor("w2c", [D, P], bf16, kind="ExternalInput").ap()
    invc = nc.dram_tensor("invc", [D, GCP], f32, kind="ExternalInput").ap()
    iotac = nc.dram_tensor("iotac", [P, T, WIN], bf16, kind="ExternalInput").ap()
    identc = nc.dram_tensor("identc", [P, P], f32, kind="ExternalInput").ap()
    out = nc.dram_tensor("out", [GCP, D], f32, kind="ExternalOutput").ap()

    with tile.TileContext(nc) as tc, ExitStack() as ctx:
        cpool = ctx.enter_context(tc.tile_pool(name="const", bufs=1))
        px = ctx.enter_context(tc.tile_pool(name="px", bufs=8))
        pxt = ctx.enter_context(tc.tile_pool(name="pxt", bufs=5))
        pbe = ctx.enter_context(tc.tile_pool(name="pbe", bufs=3))
        pm = ctx.enter_context(tc.tile_pool(name="pm", bufs=5))
        pmp = ctx.enter_context(tc.tile_pool(name="pmp", bufs=3))
        psg = ctx.enter_context(tc.tile_pool(name="psg", bufs=3))
        ptg = ctx.enter_context(tc.tile_pool(name="ptg", bufs=3))
        pmid = ctx.enter_context(tc.tile_pool(name="pmid", bufs=2))
        pk = ctx.enter_context(tc.tile_pool(name="pk", bufs=3))
        pp1 = ctx.enter_context(tc.tile_pool(name="pp1", bufs=3, space="PSUM"))
        ppd = ctx.enter_context(tc.tile_pool(name="ppd", bufs=4, space="PSUM"))
        ppw = ctx.enter_context(tc.tile_pool(name="ppw", bufs=1, space="PSUM"))

        iota_sb = cpool.tile([P, T, WIN], bf16)
        nc.sync.dma_start(iota_sb[:], iotac[:])
        ident_sb = cpool.tile([P, P], f32)
        nc.sync.dma_start(ident_sb[:], identc[:])
        w2_sb = cpool.tile([D, P], bf16)
        nc.sync.dma_start(w2_sb[:], w2c[:])
        inv_sb = cpool.tile([D, GCP], f32)
        nc.sync.dma_start(inv_sb[:], invc[:])
        c0_sb = cpool.tile([1, n_meg], i32)
        nc.sync.dma_start(c0_sb[:], c0s[:])

        acc1 = cpool.tile([D, GCP], f32)
        nc.vector.memset(acc1[:], 0.0)
        acc2 = cpool.tile([D, GCP], f32)
        nc.vector.memset(acc2[:], 0.0)

        def c0_of(t, engines):
            return nc.values_load(
                c0_sb[0:1, t:t + 1], engines=engines,
                min_val=0, max_val=GCP - WIN, skip_runtime_bounds_check=True)

        xs_ts = [None] * n_meg
        xt_ts = [None] * n_meg
        m_ts = [None] * n_meg
        cv_ts = [None] * n_meg

        def S1(t):
            xs_t = px.tile([P, T, DP2], bf16, tag="xs")
            nc.sync.dma_start(xs_t[:], xs[t])
            xts_t = pxt.tile([P, T // 2, P], fp8, tag="xts")
            nc.gpsimd.dma_start(xts_t[:], xts[t])
            b32e = pbe.tile([P, T, WIN], bf16, tag="b32e")
            nc.scalar.copy(b32e[:], xs_t[:, :, D + 1].to_broadcast([P, T, WIN]))
            m = pm.tile([P, T, WIN], bf16, tag="M")
            nc.vector.tensor_tensor(out=m[:], in0=iota_sb[:], in1=b32e[:],
                                    op=ALU.is_equal)
            ps1 = pp1.tile([P, WIN], f32, tag="pacc")
            for jj in range(T // 2):
                nc.tensor.matmul(ps1[0:D, :], lhsT=xs_t[:, 2 * jj, 0:D],
                                 rhs=m[:, 2 * jj, :], tile_position=(0, 0),
                                 start=(jj == 0), stop=(jj == T // 2 - 1))
                nc.tensor.matmul(ps1[D:P, :], lhsT=xs_t[:, 2 * jj + 1, 0:D],
                                 rhs=m[:, 2 * jj + 1, :], tile_position=(0, 64),
                                 start=(jj == 0), stop=(jj == T // 2 - 1))
            cv = c0_of(t, engines=[ENG.DVE])
            a = acc1[:, bass.ds(cv, WIN)]
            nc.vector.tensor_tensor(out=a, in0=a, in1=ps1[0:D, :], op=ALU.add)
            nc.vector.tensor_tensor(out=a, in0=a, in1=ps1[D:P, :], op=ALU.add)
            xs_ts[t], xt_ts[t], m_ts[t], cv_ts[t] = xs_t, xts_t, m, cv

        def MID(t):
            cv = cv_ts[t]
            meanT = pmid.tile([D, WIN], bf16, tag="meanT")
            nc.vector.tensor_tensor(out=meanT[:],
                                    in0=acc1[:, bass.ds(cv, WIN)],
                                    in1=inv_sb[:, bass.ds(cv, WIN)],
                                    op=ALU.mult)
            psW = ppw.tile([P, WIN], f32, tag="mid")
            nc.tensor.matmul(psW[:], lhsT=w2_sb[:], rhs=meanT[:],
                             start=True, stop=True)
            tgwin = ptg.tile([P, 2, WIN], bf16, tag="tgwin")
            nc.vector.memset(tgwin[:], 0.0)
            nc.scalar.activation(tgwin[0:D, 0, :], psW[0:D, :], AF.Tanh)
            nc.scalar.activation(tgwin[D:P, 1, :], psW[D:P, :], AF.Tanh)
            return tgwin

        def DOTS(t, tgwin):
            xts_t, m = xt_ts[t], m_ts[t]
            sg = psg.tile([P, T, WIN], bf16, tag="sg")
            for h in range(2):
                psD = ppd.tile([P, T // 2, WIN], f32, tag="psD")
                for j2 in range(T // 4):
                    jj = h * (T // 4) + j2
                    nc.tensor.matmul(psD[:, 2 * j2:2 * j2 + 2, :],
                                     lhsT=xts_t[:, jj, :],
                                     rhs=tgwin[:, :, :],
                                     start=True, stop=True)
                nc.scalar.activation(sg[:, h * (T // 2):(h + 1) * (T // 2), :],
                                     psD[:], AF.Sigmoid)
            mp = pmp.tile([P, T, WIN], bf16, tag="Mp")
            nc.vector.tensor_tensor(out=mp[:], in0=m[:], in1=sg[:],
                                    op=ALU.mult)
            return mp

        def S2(t, mp):
            xs_t, cv = xs_ts[t], cv_ts[t]
            ps3 = pp1.tile([P, WIN], f32, tag="pacc")
            for jj in range(T // 2):
                nc.tensor.matmul(ps3[0:D, :], lhsT=xs_t[:, 2 * jj, 0:D],
                                 rhs=mp[:, 2 * jj, :], tile_position=(0, 0),
                                 start=(jj == 0), stop=(jj == T // 2 - 1))
                nc.tensor.matmul(ps3[D:P, :], lhsT=xs_t[:, 2 * jj + 1, 0:D],
                                 rhs=mp[:, 2 * jj + 1, :], tile_position=(0, 64),
                                 start=(jj == 0), stop=(jj == T // 2 - 1))
            a = acc2[:, bass.ds(cv_ts[t], WIN)]
            nc.vector.tensor_tensor(out=a, in0=a, in1=ps3[0:D, :], op=ALU.add)
            nc.vector.tensor_tensor(out=a, in0=a, in1=ps3[D:P, :], op=ALU.add)
            xs_ts[t] = xt_ts[t] = m_ts[t] = cv_ts[t] = None

        # ---------------- fused pipeline (lag-2: PE never waits) ------------
        S1(0)
        if n_meg > 1:
            S1(1)
        mp_prev = None
        for t in range(n_meg):
            tgwin = MID(t)
            if t + 2 < n_meg:
                S1(t + 2)
            mp = DOTS(t, tgwin)
            if mp_prev is not None:
                S2(t - 1, mp_prev)
            mp_prev = mp
        S2(n_meg - 1, mp_prev)

        # ---------------- end: transpose acc2 -> out ------------------------
        for c in range(NCHUNK):
            pst = ppw.tile([P, D], f32, tag="mid")
            nc.tensor.transpose(pst[:], acc2[0:D, c * P:(c + 1) * P],
                                ident_sb[0:D, 0:D])
            oc = pk.tile([P, D], f32, tag="oc")
            nc.scalar.copy(oc[:], pst[:])
            nc.gpsimd.dma_start(out[c * P:(c + 1) * P, :], oc[:])

    nc.compile()
    return nc


# ----------------------------------------------------------------------------
# entry point
# ----------------------------------------------------------------------------

_CACHE = {}


def kernel(x, batch, size, W):
    global LAST_EXEC_NS
    from concourse import bass_utils

    x = np.asarray(x, dtype=np.float32)
    batch_np = np.asarray(batch).astype(np.int64)
    W = np.asarray(W, dtype=np.float32)
    size = int(size)
    cores = CORES

    gsplit, nsplit, counts = _shard_plan(batch_np, size, cores)
    max_nodes = max(nsplit[k + 1] - nsplit[k] for k in range(cores))
    n_meg = max(2, -(-max_nodes // NT))

    iota, ident, w2 = _host_consts(W)
    in_maps = []
    gcs = []
    for k in range(cores):
        m, gc = _prep_core(x, batch_np, counts, gsplit[k], gsplit[k + 1],
                           nsplit[k], nsplit[k + 1], n_meg)
        m["w2c"] = w2
        m["iotac"] = iota
        m["identc"] = ident
        in_maps.append(m)
        gcs.append(gc)

    if n_meg not in _CACHE:
        _CACHE[n_meg] = build_nc(n_meg)
    nc = _CACHE[n_meg]

    trace = os.environ.get("BASS_KERNEL_TRACE", "0") == "1"
    res = bass_utils.run_bass_kernel_spmd(nc, in_maps,
                                          core_ids=list(range(cores)),
                                          trace=trace)
    LAST_EXEC_NS = res.exec_time_ns
    outs = [res.results[k]["out"][:gcs[k]] for k in range(cores)]
    full = np.concatenate(outs, axis=0)
    if full.shape[0] < size:
        full = np.concatenate(
            [full, np.zeros((size - full.shape[0], D), np.float32)], axis=0)
    return np.ascontiguousarray(full[:size], dtype=np.float32)


# revision 18
# speedup vs baseline: 1.4325x; 1.0025x over previous
"""Trainium2 Bass kernel for the GNN attention module
(scatter-mean -> dense+tanh -> attention coefs -> weighted scatter-add),
data-parallel over graphs on 8 NeuronCores.

Self-contained: hardcodes N=2000000, D=64, G=8192, 8 cores.

v4 (baseline structure + three surgical changes):
  - xts (transposed x for the dot-products) is fp8_e4m3: halves the
    second read of x (HBM 66.7MB -> 50.4MB per core-set). fp8
    [128,128] stationaries keep FWL eligibility.
  - sigmoid is applied during PSUM evacuation of the all-window dots
    (ACT reads PSUM, writes bf16 SBUF); the one-hot pick becomes a
    cheap bf16 multiply mp = m * sigma(psD) (exact: m is one-hot).
    Replaces the scalar_tensor_tensor saturation trick + f16 sigmoid.
  - the one-hot m is built on the (otherwise idle) gpsimd engine via a
    broadcast-read is_equal; xts streams on the gpsimd SWDGE queue
    while xs streams on the sync HWDGE queue.
Seg-sum matmuls keep the baseline shape: x as the [128,64] stationary
per block, one-hot as the 32-col moving operand, col-tiled pairs at
(0,0)/(0,64), accumulated in [128, WIN] psum, added into acc [64, GCP].
"""
import os
import numpy as np
from contextlib import ExitStack

import ml_dtypes

P = 128          # partitions / nodes per block
T = 32           # blocks per mega-tile
NT = P * T       # nodes per mega-tile (4096)
WIN = 32         # mega window width (graphs)
D = 64
DP2 = D + 2      # x columns + pad + packed per-block graph offset (b32)
N_FULL = 2_000_000
G_FULL = 8192
CORES = 8
GCP = 1152       # padded local graph count (9 * 128)
NCHUNK = GCP // P

LAST_EXEC_NS = None


# ----------------------------------------------------------------------------
# host-side preprocessing
# ----------------------------------------------------------------------------

def _shard_plan(batch, size, cores):
    counts = np.bincount(batch.astype(np.int64), minlength=size)
    cum = np.concatenate([[0], np.cumsum(counts)])
    n = batch.shape[0]
    gsplit = [0]
    for k in range(1, cores):
        g = int(np.searchsorted(cum, k * n / cores))
        g = max(gsplit[-1] + 1, min(g, size - (cores - k)))
        gsplit.append(g)
    gsplit.append(size)
    nsplit = [int(cum[g]) for g in gsplit]
    return gsplit, nsplit, counts


def _prep_core(x, batch, counts, g0, g1, n0, n1, n_meg):
    nn = n1 - n0
    npad = n_meg * NT
    lg = (batch[n0:n1] - g0).astype(np.int64)
    gc = g1 - g0
    ghost = gc                           # pad nodes get this local graph id
    lg_full = np.full(npad, ghost, dtype=np.int64)
    lg_full[:nn] = lg

    xs_pad = np.zeros((npad, D), dtype=np.float32)
    xs_pad[:nn] = x[n0:n1]

    lgt = lg_full.reshape(n_meg, T, P)            # [t, j, p]
    c0 = np.minimum(lgt[:, 0, 0], GCP - WIN)      # mega window base
    b32 = lgt - c0[:, None, None]
    assert b32.min() >= 0 and b32.max() < WIN, (b32.min(), b32.max())
    assert ghost + 1 <= GCP

    b32 = b32.transpose(0, 2, 1).astype(np.float32)   # [t, p, j]

    xs4 = np.zeros((n_meg, P, T, DP2), dtype=np.float32)
    xs4[:, :, :, :D] = xs_pad.reshape(n_meg, T, P, D).transpose(0, 2, 1, 3)
    xs4 = xs4.astype(ml_dtypes.bfloat16)
    xs4[:, :, :, D + 1] = b32.astype(ml_dtypes.bfloat16)
    xtb = xs_pad.reshape(n_meg, T, P, D).transpose(0, 1, 3, 2)  # [t, j, d, q]
    xts = np.ascontiguousarray(
        xtb.reshape(n_meg, T // 2, 2, D, P).transpose(0, 2, 3, 1, 4)
        .reshape(n_meg, P, (T // 2) * P)
    ).astype(ml_dtypes.float8_e4m3)

    c0s = np.zeros((1, n_meg), dtype=np.int32)
    c0s[0, :] = c0

    # 1 / max(counts, 1) for the local graphs, expanded to 64 partitions
    lc = np.ones(GCP, dtype=np.float64)
    lc[:gc] = np.maximum(counts[g0:g1], 1)
    inv = (1.0 / lc).astype(np.float32)
    invexp = np.ascontiguousarray(np.broadcast_to(inv, (D, GCP)))
    return {"xs": xs4, "xts": xts, "c0s": c0s, "invc": invexp}, gc


def _host_consts(W):
    iota = np.broadcast_to(
        np.arange(WIN, dtype=np.float32), (P, T, WIN)
    ).astype(ml_dtypes.bfloat16)
    ident = np.eye(P, dtype=np.float32)
    w2 = np.ascontiguousarray(
        np.tile(W.astype(np.float32), (1, 2))
    ).astype(ml_dtypes.bfloat16)          # [64, 128] = [W | W]
    return iota, ident, w2


# ----------------------------------------------------------------------------
# device kernel
# ----------------------------------------------------------------------------

def build_nc(n_meg):
    from concourse import mybir
    import concourse.tile as tile
    import concourse.bacc as bacc
    import concourse.bass as bass

    f32 = mybir.dt.float32
    bf16 = mybir.dt.bfloat16
    fp8 = mybir.dt.float8e4
    i32 = mybir.dt.int32
    AF = mybir.ActivationFunctionType
    ALU = mybir.AluOpType
    ENG = mybir.EngineType

    nc = bacc.Bacc("TRN2", target_bir_lowering=False, debug=False,
                   num_devices=CORES)

    xs = nc.dram_tensor("xs", [n_meg, P, T, DP2], bf16, kind="ExternalInput").ap()
    xts = nc.dram_tensor("xts", [n_meg, P, (T // 2) * P], fp8, kind="ExternalInput").ap()
    c0s = nc.dram_tensor("c0s", [1, n_meg], i32, kind="ExternalInput").ap()
    w2c = nc.dram_tensor("w2c", [D, P], bf16, kind="ExternalInput").ap()
    invc = nc.dram_tensor("invc", [D, GCP], f32, kind="ExternalInput").ap()
    iotac = nc.dram_tensor("iotac", [P, T, WIN], bf16, kind="ExternalInput").ap()
    identc = nc.dram_tensor("identc", [P, P], f32, kind="ExternalInput").ap()
    out = nc.dram_tensor("out", [GCP, D], f32, kind="ExternalOutput").ap()

    with tile.TileContext(nc) as tc, ExitStack() as ctx:
        cpool = ctx.enter_context(tc.tile_pool(name="const", bufs=1))
        px = ctx.enter_context(tc.tile_pool(name="px", bufs=8))
        pxt = ctx.enter_context(tc.tile_pool(name="pxt", bufs=5))
        pbe = ctx.enter_context(tc.tile_pool(name="pbe", bufs=3))
        pm = ctx.enter_context(tc.tile_pool(name="pm", bufs=5))
        pmp = ctx.enter_context(tc.tile_pool(name="pmp", bufs=3))
        psg = ctx.enter_context(tc.tile_pool(name="psg", bufs=3))
        ptg = ctx.enter_context(tc.tile_pool(name="ptg", bufs=3))
        pmid = ctx.enter_context(tc.tile_pool(name="pmid", bufs=2))
        pk = ctx.enter_context(tc.tile_pool(name="pk", bufs=3))
        pp1 = ctx.enter_context(tc.tile_pool(name="pp1", bufs=3, space="PSUM"))
        ppd = ctx.enter_context(tc.tile_pool(name="ppd", bufs=4, space="PSUM"))
        ppw = ctx.enter_context(tc.tile_pool(name="ppw", bufs=1, space="PSUM"))

        iota_sb = cpool.tile([P, T, WIN], bf16)
        nc.gpsimd.dma_start(iota_sb[:], iotac[:])
        ident_sb = cpool.tile([P, P], f32)
        nc.gpsimd.dma_start(ident_sb[:], identc[:])
        w2_sb = cpool.tile([D, P], bf16)
        nc.gpsimd.dma_start(w2_sb[:], w2c[:])
        inv_sb = cpool.tile([D, GCP], f32)
        nc.gpsimd.dma_start(inv_sb[:], invc[:])
        c0_sb = cpool.tile([1, n_meg], i32)
        nc.gpsimd.dma_start(c0_sb[:], c0s[:])

        acc1 = cpool.tile([D, GCP], f32)
        nc.vector.memset(acc1[:], 0.0)
        acc2 = cpool.tile([D, GCP], f32)
        nc.vector.memset(acc2[:], 0.0)

        def c0_of(t, engines):
            return nc.values_load(
                c0_sb[0:1, t:t + 1], engines=engines,
                min_val=0, max_val=GCP - WIN, skip_runtime_bounds_check=True)

        xs_ts = [None] * n_meg
        xt_ts = [None] * n_meg
        m_ts = [None] * n_meg
        cv_ts = [None] * n_meg

        def S1(t):
            xs_t = px.tile([P, T, DP2], bf16, tag="xs")
            nc.sync.dma_start(xs_t[:], xs[t])
            xts_t = pxt.tile([P, T // 2, P], fp8, tag="xts")
            nc.gpsimd.dma_start(xts_t[:], xts[t])
            b32e = pbe.tile([P, T, WIN], bf16, tag="b32e")
            nc.scalar.copy(b32e[:], xs_t[:, :, D + 1].to_broadcast([P, T, WIN]))
            m = pm.tile([P, T, WIN], bf16, tag="M")
            nc.vector.tensor_tensor(out=m[:], in0=iota_sb[:], in1=b32e[:],
                                    op=ALU.is_equal)
            ps1 = pp1.tile([P, WIN], f32, tag="pacc")
            for jj in range(T // 2):
                nc.tensor.matmul(ps1[0:D, :], lhsT=xs_t[:, 2 * jj, 0:D],
                                 rhs=m[:, 2 * jj, :], tile_position=(0, 0),
                                 start=(jj == 0), stop=(jj == T // 2 - 1))
                nc.tensor.matmul(ps1[D:P, :], lhsT=xs_t[:, 2 * jj + 1, 0:D],
                                 rhs=m[:, 2 * jj + 1, :], tile_position=(0, 64),
                                 start=(jj == 0), stop=(jj == T // 2 - 1))
            cv = c0_of(t, engines=[ENG.DVE])
            a = acc1[:, bass.ds(cv, WIN)]
            nc.vector.tensor_tensor(out=a, in0=a, in1=ps1[0:D, :], op=ALU.add)
            nc.vector.tensor_tensor(out=a, in0=a, in1=ps1[D:P, :], op=ALU.add)
            xs_ts[t], xt_ts[t], m_ts[t], cv_ts[t] = xs_t, xts_t, m, cv

        def MEANT(t):
            cv = cv_ts[t]
            meanT = pmid.tile([D, WIN], bf16, tag="meanT")
            nc.vector.tensor_tensor(out=meanT[:],
                                    in0=acc1[:, bass.ds(cv, WIN)],
                                    in1=inv_sb[:, bass.ds(cv, WIN)],
                                    op=ALU.mult)
            return meanT

        def MIDW(t, meanT):
            psW = ppw.tile([P, WIN], f32, tag="mid")
            nc.tensor.matmul(psW[:], lhsT=w2_sb[:], rhs=meanT[:],
                             start=True, stop=True)
            tgwin = ptg.tile([P, 2, WIN], bf16, tag="tgwin")
            nc.vector.memset(tgwin[:], 0.0)
            nc.scalar.activation(tgwin[0:D, 0, :], psW[0:D, :], AF.Tanh)
            nc.scalar.activation(tgwin[D:P, 1, :], psW[D:P, :], AF.Tanh)
            return tgwin

        def DOTS(t, tgwin):
            xts_t, m = xt_ts[t], m_ts[t]
            sg = psg.tile([P, T, WIN], bf16, tag="sg")
            for h in range(2):
                psD = ppd.tile([P, T // 2, WIN], f32, tag="psD")
                for j2 in range(T // 4):
                    jj = h * (T // 4) + j2
                    nc.tensor.matmul(psD[:, 2 * j2:2 * j2 + 2, :],
                                     lhsT=xts_t[:, jj, :],
                                     rhs=tgwin[:, :, :],
                                     start=True, stop=True)
                nc.scalar.activation(sg[:, h * (T // 2):(h + 1) * (T // 2), :],
                                     psD[:], AF.Sigmoid)
            mp = pmp.tile([P, T, WIN], bf16, tag="Mp")
            nc.vector.tensor_tensor(out=mp[:], in0=m[:], in1=sg[:],
                                    op=ALU.mult)
            return mp

        def S2(t, mp):
            xs_t, cv = xs_ts[t], cv_ts[t]
            ps3 = pp1.tile([P, WIN], f32, tag="pacc")
            for jj in range(T // 2):
                nc.tensor.matmul(ps3[0:D, :], lhsT=xs_t[:, 2 * jj, 0:D],
                                 rhs=mp[:, 2 * jj, :], tile_position=(0, 0),
                                 start=(jj == 0), stop=(jj == T // 2 - 1))
                nc.tensor.matmul(ps3[D:P, :], lhsT=xs_t[:, 2 * jj + 1, 0:D],
                                 rhs=mp[:, 2 * jj + 1, :], tile_position=(0, 64),
                                 start=(jj == 0), stop=(jj == T // 2 - 1))
            a = acc2[:, bass.ds(cv_ts[t], WIN)]
            nc.vector.tensor_tensor(out=a, in0=a, in1=ps3[0:D, :], op=ALU.add)
            nc.vector.tensor_tensor(out=a, in0=a, in1=ps3[D:P, :], op=ALU.add)
            xs_ts[t] = xt_ts[t] = m_ts[t] = cv_ts[t] = None

        # ---------------- fused pipeline (lag-2: PE never waits) ------------
        S1(0)
        if n_meg > 1:
            S1(1)
        mean_prev = MEANT(0)
        mp_prev = None
        for t in range(n_meg):
            tgwin = MIDW(t, mean_prev)
            if t + 2 < n_meg:
                S1(t + 2)
            if t + 1 < n_meg:
                mean_prev = MEANT(t + 1)
            mp = DOTS(t, tgwin)
            if mp_prev is not None:
                S2(t - 1, mp_prev)
            mp_prev = mp
        S2(n_meg - 1, mp_prev)

        # ---------------- end: transpose acc2 -> out ------------------------
        for c in range(NCHUNK):
            pst = ppw.tile([P, D], f32, tag="mid")
            nc.tensor.transpose(pst[:], acc2[0:D, c * P:(c + 1) * P],
                                ident_sb[0:D, 0:D])
            oc = pk.tile([P, D], f32, tag="oc")
            nc.scalar.copy(oc[:], pst[:])
            nc.gpsimd.dma_start(out[c * P:(c + 1) * P, :], oc[:])

    nc.compile()
    return nc


# ----------------------------------------------------------------------------
# entry point
# ----------------------------------------------------------------------------

_CACHE = {}


def kernel(x, batch, size, W):
    global LAST_EXEC_NS
    from concourse import bass_utils

    x = np.asarray(x, dtype=np.float32)
    batch_np = np.asarray(batch).astype(np.int64)
    W = np.asarray(W, dtype=np.float32)
    size = int(size)
    cores = CORES

    gsplit, nsplit, counts = _shard_plan(batch_np, size, cores)
    max_nodes = max(nsplit[k + 1] - nsplit[k] for k in range(cores))
    n_meg = max(2, -(-max_nodes // NT))

    iota, ident, w2 = _host_consts(W)
    in_maps = []
    gcs = []
    for k in range(cores):
        m, gc = _prep_core(x, batch_np, counts, gsplit[k], gsplit[k + 1],
                           nsplit[k], nsplit[k + 1], n_meg)
        m["w2c"] = w2
        m["iotac"] = iota
        m["identc"] = ident
        in_maps.append(m)
        gcs.append(gc)

    if n_meg not in _CACHE:
        _CACHE[n_meg] = build_nc(n_meg)
    nc = _CACHE[n_meg]

    trace = os.environ.get("BASS_KERNEL_TRACE", "0") == "1"
    res = bass_utils.run_bass_kernel_spmd(nc, in_maps,
                                          core_ids=list(range(cores)),
                                          trace=trace)
    LAST_EXEC_NS = res.exec_time_ns
    outs = [res.results[k]["out"][:gcs[k]] for k in range(cores)]
    full = np.concatenate(outs, axis=0)
    if full.shape[0] < size:
        full = np.concatenate(
            [full, np.zeros((size - full.shape[0], D), np.float32)], axis=0)
    return np.ascontiguousarray(full[:size], dtype=np.float32)
